# revision 17
# baseline (speedup 1.0000x reference)
"""Trainium2 Bass kernel for a 2-layer GRU (Keras reset_after) + 3 Dense layers.

Model (per reference):
  h1 = GRU(x; k1, r1, b1)            # [B,T,64] -> [B,T,256], full sequence
  h2 = GRU(h1; k2, r2, b2)[:, -1]    # last state, [B,128]
  y  = ((h2 @ w3 + b3) @ w4 + b4) @ w5 + b5   # [B,24]

Strategy: pure data parallel over 8 NeuronCores (batch 256 -> 32 per core).
Everything on-chip per core; transposed layout (units on partitions, batch on
the free dim) so the sequential scan needs no per-step transposes.

The scan is dependency-latency bound, so the schedule minimizes the serial
chain per step, and every PSUM accumulator gets a full 2KB bank to itself so
the framework's bank-level dependency tracking never manufactures a false
cross-step wait:
  - r gates accumulate in their own bank; a sigmoid over just the r columns
    gates the candidate path well before the z sigmoid is needed.
  - GRU2's projections that depend only on h1 (input kernel + biases) are
    emitted early; the h2-recurrent matmuls are emitted last in the PE
    stream so their wait on the previous h2 never blocks GRU1's matmuls.
  - GRU1's lerp is rewritten h' = u + w*hh with u = h - w*h computed on the
    (otherwise idle) GpSimd engine during the tanh window; only two DVE ops
    remain after each tanh.
  - h1 is written in two 32-col halves so the next step's k=0 matmuls start
    before the k=1 half lands.
  - z-gate columns of all weights are negated on the host so one sigmoid
    yields w = 1 - z directly.

All matmul weights/activations fp16 (fp32 PSUM accumulate), gate math fp32
internally on DVE/ACT with fp16 storage.
"""

import numpy as np

import concourse.bass as bass
import concourse.mybir as mybir
import concourse.tile as tile
from concourse import bacc
from concourse.bass_utils import run_bass_kernel_spmd

F16 = mybir.dt.float16
F32 = mybir.dt.float32
AF = mybir.ActivationFunctionType
OP = mybir.AluOpType

B, T_FULL, F = 256, 512, 64
U1, U2, OUT = 256, 128, 24
NCORES = 8
BL = B // NCORES  # 32 local batch


def _prep(inputs, T):
    """Host-side preprocessing -> (list of per-core input dicts, flags)."""
    x = np.asarray(inputs["x"], np.float32)[:, :T, :]
    k1 = np.asarray(inputs["k1"], np.float32)
    r1 = np.asarray(inputs["r1"], np.float32)
    b1 = np.asarray(inputs["b1"], np.float32)
    k2 = np.asarray(inputs["k2"], np.float32)
    r2 = np.asarray(inputs["r2"], np.float32)
    b2 = np.asarray(inputs["b2"], np.float32)
    w3 = np.asarray(inputs["w3"], np.float32)
    b3 = np.asarray(inputs["b3"], np.float32)
    w4 = np.asarray(inputs["w4"], np.float32)
    b4 = np.asarray(inputs["b4"], np.float32)
    w5 = np.asarray(inputs["w5"], np.float32)
    b5 = np.asarray(inputs["b5"], np.float32)

    s1 = np.ones(3 * U1, np.float32)
    s1[:U1] = -1.0  # z-gate negation
    s2 = np.ones(3 * U2, np.float32)
    s2[:U2] = -1.0

    k1e = k1 * s1
    r1e = r1 * s1
    b1e = (b1[0] + b1[1]) * s1  # only used for z/r columns
    k2e = k2 * s2
    r2e = r2 * s2
    b2zr = ((b2[0] + b2[1]) * s2)[: 2 * U2]

    # k1 z/r part augmented with bias row: [65, 512]
    wk1zr = np.concatenate([k1e[:, : 2 * U1], b1e[None, : 2 * U1]], 0)
    # k1 h part augmented with input-bias row: [65, 256]
    wk1h = np.concatenate([k1[:, 2 * U1 :], b1[0][None, 2 * U1 :]], 0)
    # r1 tiles: tile (m,k) at cols (m*2+k)*128
    wr1 = r1e.reshape(2, 128, 6, 128).transpose(1, 2, 0, 3).reshape(128, 12 * 128)
    # k2 tiles: tile (m,k) at cols (m*2+k)*128
    wk2 = k2e.reshape(2, 128, 3, 128).transpose(1, 2, 0, 3).reshape(128, 6 * 128)
    wr2 = r2e  # [128, 384], tile m at m*128

    vb1h = np.stack([b1[1, 2 * U1 : 2 * U1 + 128], b1[1, 2 * U1 + 128 :]], 1)  # [128,2]
    vb2h = np.stack([b2[0, 2 * U2 :], b2[1, 2 * U2 :]], 1)  # [128,2]
    vbd = np.zeros((128, 3), np.float32)
    vbd[:64, 0] = b3
    vbd[:32, 1] = b4
    vbd[:OUT, 2] = b5

    flags = {
        "HAS_B1H": bool(np.any(b1[1, 2 * U1 :] != 0)),
        "HAS_B2H": bool(np.any(b2[:, 2 * U2 :] != 0)),
    }

    shared = {
        "wk1zr": wk1zr.astype(np.float16),
        "wk1h": wk1h.astype(np.float16),
        "wr1": wr1.astype(np.float16),
        "wk2": wk2.astype(np.float16),
        "wr2": wr2.astype(np.float16),
        "wb2zr": b2zr[None, :].astype(np.float16),
        "vb1h": vb1h.astype(np.float32),
        "vb2h": vb2h.astype(np.float32),
        "vbd": vbd.astype(np.float32),
        "wd3": w3.astype(np.float16),
        "wd4": w4.astype(np.float16),
        "wd5": w5.astype(np.float16),
    }

    in_maps = []
    for c in range(NCORES):
        xs = x[c * BL : (c + 1) * BL]  # [BL, T, F]
        xt = np.ascontiguousarray(xs.transpose(2, 1, 0)).reshape(F, T * BL)
        xin = np.concatenate([xt, np.ones((1, T * BL), np.float32)], 0)
        m = dict(shared)
        m["xin"] = xin.astype(np.float16)
        in_maps.append(m)
    return in_maps, flags


def _build(T, flags):
    """Emit the Bass program for T timesteps. Returns compiled nc."""
    HAS_B1H = flags["HAS_B1H"]
    HAS_B2H = flags["HAS_B2H"]
    nc = bacc.Bacc("TRN2", target_bir_lowering=False, debug=False, num_devices=NCORES)

    d_xin = nc.dram_tensor("xin", [F + 1, T * BL], F16, kind="ExternalInput").ap()
    d_wk1zr = nc.dram_tensor("wk1zr", [F + 1, 512], F16, kind="ExternalInput").ap()
    d_wk1h = nc.dram_tensor("wk1h", [F + 1, 256], F16, kind="ExternalInput").ap()
    d_wr1 = nc.dram_tensor("wr1", [128, 1536], F16, kind="ExternalInput").ap()
    d_wk2 = nc.dram_tensor("wk2", [128, 768], F16, kind="ExternalInput").ap()
    d_wr2 = nc.dram_tensor("wr2", [128, 384], F16, kind="ExternalInput").ap()
    d_wb2zr = nc.dram_tensor("wb2zr", [1, 256], F16, kind="ExternalInput").ap()
    d_vb1h = nc.dram_tensor("vb1h", [128, 2], F32, kind="ExternalInput").ap()
    d_vb2h = nc.dram_tensor("vb2h", [128, 2], F32, kind="ExternalInput").ap()
    d_vbd = nc.dram_tensor("vbd", [128, 3], F32, kind="ExternalInput").ap()
    d_wd3 = nc.dram_tensor("wd3", [128, 64], F16, kind="ExternalInput").ap()
    d_wd4 = nc.dram_tensor("wd4", [64, 32], F16, kind="ExternalInput").ap()
    d_wd5 = nc.dram_tensor("wd5", [32, OUT], F16, kind="ExternalInput").ap()
    d_y = nc.dram_tensor("y", [BL, OUT], F32, kind="ExternalOutput").ap()

    with tile.TileContext(nc) as tc:
        with (
            tc.tile_pool(name="big", bufs=1) as big,
            tc.tile_pool(name="wts", bufs=1) as wts,
            tc.tile_pool(name="state", bufs=1) as state,
            tc.tile_pool(name="tmp", bufs=3) as tmp,
        ):
            sb_x = big.tile([F + 1, T * BL], F16, tag="sb_x", name="sb_x")
            # flat [128, T*64] so the per-step slice is a clean 2-D AP (a 3-D
            # AP on the DVE costs ~2x on the critical-path add)
            sb_xg1h = big.tile([128, T * 64], F16, tag="sb_xg1h", name="sb_xg1h")
            xg1h_3d = sb_xg1h.rearrange("p (t c) -> p t c", c=64)

            def wtile(name, shape, dt, src):
                t_ = wts.tile(shape, dt, tag=name, name=name)
                nc.sync.dma_start(out=t_[:], in_=src[:])
                return t_

            sb_wk1zr = wtile("sb_wk1zr", [F + 1, 512], F16, d_wk1zr)
            sb_wk1h = wtile("sb_wk1h", [F + 1, 256], F16, d_wk1h)
            sb_wr1 = wtile("sb_wr1", [128, 1536], F16, d_wr1)
            sb_wk2 = wtile("sb_wk2", [128, 768], F16, d_wk2)
            sb_wr2 = wtile("sb_wr2", [128, 384], F16, d_wr2)
            sb_wb2zr = wtile("sb_wb2zr", [1, 256], F16, d_wb2zr)
            sb_vb1h = wtile("sb_vb1h", [128, 2], F32, d_vb1h)
            sb_vb2h = wtile("sb_vb2h", [128, 2], F32, d_vb2h)
            sb_vbd = wtile("sb_vbd", [128, 3], F32, d_vbd)
            sb_wd3 = wtile("sb_wd3", [128, 64], F16, d_wd3)
            sb_wd4 = wtile("sb_wd4", [64, 32], F16, d_wd4)
            sb_wd5 = wtile("sb_wd5", [32, OUT], F16, d_wd5)

            sb_ones = wts.tile([1, BL], F16, tag="sb_ones", name="sb_ones")
            nc.vector.memset(sb_ones[:], 1.0)

            # x load, split across a few DMAs
            nchunk = 4
            cw = (T * BL) // nchunk
            for i in range(nchunk):
                nc.sync.dma_start(
                    out=sb_x[:, i * cw : (i + 1) * cw],
                    in_=d_xin[:, i * cw : (i + 1) * cw],
                )

            # ---- bulk precompute xg1h = [x;1] @ [k1_h; b1_0h]  -> sb_xg1h ----
            with tc.tile_pool(name="bulkps", bufs=4, space="PSUM") as bulkps:
                CH = 16  # timesteps per matmul (N = CH*BL = 512)
                for ci in range((T + CH - 1) // CH):
                    t0 = ci * CH
                    ts_ = min(CH, T - t0)
                    n = ts_ * BL
                    for m in range(2):
                        pb = bulkps.tile([128, 512], F32, tag="pb", name="pb")
                        nc.tensor.matmul(
                            pb[:, :n],
                            sb_wk1h[:, m * 128 : (m + 1) * 128],
                            sb_x[:, t0 * BL : t0 * BL + n],
                            start=True,
                            stop=True,
                        )
                        dst = xg1h_3d[:, t0 : t0 + ts_, m * 32 : (m + 1) * 32]
                        src = pb.rearrange("p (t b) -> p t b", b=BL)[:, :ts_, :]
                        if m == 0:
                            nc.vector.tensor_copy(dst, src)
                        else:
                            nc.scalar.copy(dst, src)

            # ---- the scan ----
            # Eight PSUM accumulators, each padded to a full 2KB bank so the
            # framework's dependency tracking (bank-granular) never couples
            # them:
            #   ps_r[i]  cols 0:64   gru1 r gates (m=2 -> 0:32, m=3 -> 32:64)
            #   ps_z[i]  cols 0:64   gru1 z gates (m=0 -> 0:32, m=1 -> 32:64)
            #   ps_h[i]  cols 0:64   gru1 candidate (m=4 -> 0:32, m=5 -> 32:64)
            #   ps_g[i]  cols 0:64 gru2 z/r gates; 64:96 xh2; 96:128 rh2
            with tc.tile_pool(name="ps", bufs=1, space="PSUM") as psp:
                def pbank(nm):
                    return [
                        psp.tile([128, 512], F32, tag=f"{nm}_{i}", name=f"{nm}_{i}")
                        for i in range(2)
                    ]

                ps_r = pbank("ps_r")
                ps_z = pbank("ps_z")
                ps_h = pbank("ps_h")
                ps_g = pbank("ps_g")
                sb_h1 = [
                    state.tile([128, 64], F16, tag=f"sb_h1_{i}", name=f"sb_h1_{i}")
                    for i in range(2)
                ]
                sb_h2 = [
                    state.tile([128, BL], F16, tag=f"sb_h2_{i}", name=f"sb_h2_{i}")
                    for i in range(2)
                ]

                def emit_xg1(s):
                    """x-side z/r projections (+biases) for step s; starts the
                    accumulation groups for the r and z banks of step s."""
                    rhs = sb_x[:, s * BL : (s + 1) * BL]
                    for m in (2, 3):  # r tiles
                        nc.tensor.matmul(
                            ps_r[s % 2][:, (m - 2) * 32 : (m - 1) * 32],
                            sb_wk1zr[:, m * 128 : (m + 1) * 128],
                            rhs,
                            start=(m == 2),
                            stop=(s == 0),
                        )
                    for m in (0, 1):  # z tiles
                        nc.tensor.matmul(
                            ps_z[s % 2][:, m * 32 : (m + 1) * 32],
                            sb_wk1zr[:, m * 128 : (m + 1) * 128],
                            rhs,
                            start=(m == 0),
                            stop=(s == 0),
                        )

                def emit_rg1(t):
                    """Recurrent projections for gru1 step t: r gates first
                    (k-major so the k=0 half of h1 unblocks the first pairs),
                    then z gates, then the candidate (h) tiles."""
                    h1p = sb_h1[(t - 1) % 2]
                    for k in range(2):
                        for m in (2, 3):
                            nc.tensor.matmul(
                                ps_r[t % 2][:, (m - 2) * 32 : (m - 1) * 32],
                                sb_wr1[:, (m * 2 + k) * 128 : (m * 2 + k + 1) * 128],
                                h1p[:, k * 32 : (k + 1) * 32],
                                start=False,
                                stop=(k == 1),
                            )
                    for k in range(2):
                        for m in (0, 1):
                            nc.tensor.matmul(
                                ps_z[t % 2][:, m * 32 : (m + 1) * 32],
                                sb_wr1[:, (m * 2 + k) * 128 : (m * 2 + k + 1) * 128],
                                h1p[:, k * 32 : (k + 1) * 32],
                                start=False,
                                stop=(k == 1),
                            )
                    for k in range(2):
                        for i, m in enumerate((4, 5)):
                            nc.tensor.matmul(
                                ps_h[t % 2][:, i * 32 : (i + 1) * 32],
                                sb_wr1[:, (m * 2 + k) * 128 : (m * 2 + k + 1) * 128],
                                h1p[:, k * 32 : (k + 1) * 32],
                                start=(k == 0 and i == 0),
                                stop=(k == 1),
                            )

                def emit_gru2_early(s):
                    """gru2 projections for step s that depend only on h1[s]:
                    input-kernel z/r, biases, and xh2.  Starts the ps_g group."""
                    pg = ps_g[s % 2]
                    h1s = sb_h1[s % 2]
                    for m in range(2):  # z, r gates
                        reg = pg[:, m * 32 : (m + 1) * 32]
                        for k in range(2):
                            nc.tensor.matmul(
                                reg,
                                sb_wk2[:, (m * 2 + k) * 128 : (m * 2 + k + 1) * 128],
                                h1s[:, k * 32 : (k + 1) * 32],
                                start=(m == 0 and k == 0),
                                stop=False,
                            )
                        nc.tensor.matmul(
                            reg,
                            sb_wb2zr[:, m * 128 : (m + 1) * 128],
                            sb_ones[:],
                            start=False,
                            stop=(s == 0),
                        )
                    for k in range(2):  # xh2
                        nc.tensor.matmul(
                            pg[:, 64:96],
                            sb_wk2[:, (4 + k) * 128 : (5 + k) * 128],
                            h1s[:, k * 32 : (k + 1) * 32],
                            start=False,
                            stop=(k == 1),
                        )

                def emit_gru2_late(s):
                    """gru2 recurrent projections for step s (need h2[s-1]);
                    emitted last in the PE stream so their wait can't block
                    the next step's gru1 matmuls."""
                    pg = ps_g[s % 2]
                    h2p = sb_h2[(s - 1) % 2]
                    for m in range(2):
                        nc.tensor.matmul(
                            pg[:, m * 32 : (m + 1) * 32],
                            sb_wr2[:, m * 128 : (m + 1) * 128],
                            h2p[:],
                            start=False,
                            stop=True,
                        )
                    nc.tensor.matmul(
                        pg[:, 96:128],
                        sb_wr2[:, 256:384],
                        h2p[:],
                        start=False,
                        stop=True,
                    )

                zb_holder = [None]

                # Per step t (s = t-1 is the gru2 step) the engines run:
                #   ACT:  sig_r(t), sig_z(t), tanh1(t), sig_g2(s), tanh2(s)
                #   DVE:  t1, pre1, t2a, t2b, h1c, v2, h2c   (PSUM-capable)
                #   Pool: wh, u, v, wh2, u2                  (SBUF-only prep)
                def emit_step(t):
                    s = t - 1
                    h1p = sb_h1[(t - 1) % 2]
                    h1c = sb_h1[t % 2]

                    # -- ACT: gru1 sigmoids --
                    r1sb = tmp.tile([128, 64], F16, tag="r1sb", name="r1sb")
                    w1sb = tmp.tile([128, 64], F16, tag="w1sb", name="w1sb")
                    if t >= 1:
                        nc.scalar.activation(r1sb[:], ps_r[t % 2][:, 0:64], AF.Sigmoid)
                    nc.scalar.activation(w1sb[:], ps_z[t % 2][:, 0:64], AF.Sigmoid)

                    # -- gru1 candidate + combine --
                    hh1 = tmp.tile([128, 64], F16, tag="hh1", name="hh1")
                    if t == 0:
                        nc.scalar.activation(hh1[:], sb_xg1h[:, 0:64], AF.Tanh)
                        nc.vector.tensor_mul(h1c[:], w1sb[:], hh1[:])
                        return
                    t1b = tmp.tile([128, 64], F16, tag="t1b", name="t1b")
                    ph = ps_h[t % 2]
                    if HAS_B1H:
                        for i in range(2):
                            nc.vector.scalar_tensor_tensor(
                                t1b[:, i * 32 : (i + 1) * 32],
                                ph[:, i * 32 : (i + 1) * 32],
                                sb_vb1h[:, i : i + 1],
                                r1sb[:, i * 32 : (i + 1) * 32],
                                OP.add,
                                OP.mult,
                            )
                    else:
                        nc.vector.tensor_mul(t1b[:], ph[:, 0:64], r1sb[:])
                    pre1 = tmp.tile([128, 64], F16, tag="pre1", name="pre1")
                    nc.vector.tensor_add(
                        pre1[:], t1b[:], sb_xg1h[:, t * 64 : (t + 1) * 64]
                    )
                    # zero [128,1] written after pre1; sig_g2 takes it as its
                    # bias operand purely to pin its schedule slot after tanh1
                    # (the scheduler's cost model underestimates gru2's path
                    # and would otherwise run sig_g2 first on ACT, delaying
                    # tanh1 by ~400ns).
                    zb = tmp.tile([128, 1], F32, tag="zb", name="zb")
                    nc.vector.tensor_scalar_mul(zb[:], pre1[:, 0:1], 0.0)
                    zb_holder[0] = zb
                    nc.scalar.activation(hh1[:], pre1[:], AF.Tanh)

                    # u = (1-w)*h1p on the DVE itself, filling the tanh window
                    # (Pool is ~2x slower and contends for the SBUF port), then
                    # only two ops remain after the tanh: h1c = u + w*hh1.
                    wh = tmp.tile([128, 64], F16, tag="wh", name="wh")
                    u = tmp.tile([128, 64], F16, tag="u", name="u")
                    nc.vector.tensor_mul(wh[:], w1sb[:], h1p[:])
                    nc.vector.tensor_sub(u[:], h1p[:], wh[:])
                    v = tmp.tile([128, 64], F16, tag="v", name="v")
                    nc.vector.tensor_mul(v[:], w1sb[:], hh1[:])
                    nc.vector.tensor_add(h1c[:], u[:], v[:])

                def emit_gru2_step(s):
                    """gru2 elementwise for step s (ACT sig/tanh, DVE psum-side
                    ops + combine, Pool u2 prep)."""
                    pg = ps_g[s % 2]
                    h2p = sb_h2[(s - 1) % 2] if s > 0 else None
                    h2c = sb_h2[s % 2]
                    w2sb = tmp.tile([128, 64], F16, tag="w2sb", name="w2sb")
                    if zb_holder[0] is not None:
                        nc.scalar.activation(
                            w2sb[:], pg[:, 0:64], AF.Sigmoid, bias=zb_holder[0][:, 0:1]
                        )
                    else:
                        nc.scalar.activation(w2sb[:], pg[:, 0:64], AF.Sigmoid)

                    t2b = tmp.tile([128, BL], F16, tag="t2b", name="t2b")
                    if s == 0:
                        nc.vector.tensor_scalar_add(
                            t2b[:], pg[:, 64:96], sb_vb2h[:, 0:1]
                        )
                    else:
                        t2a = tmp.tile([128, BL], F16, tag="t2a", name="t2a")
                        if HAS_B2H:
                            nc.vector.scalar_tensor_tensor(
                                t2a[:],
                                pg[:, 96:128],
                                sb_vb2h[:, 1:2],
                                w2sb[:, 32:64],
                                OP.add,
                                OP.mult,
                            )
                            nc.vector.scalar_tensor_tensor(
                                t2b[:],
                                t2a[:],
                                sb_vb2h[:, 0:1],
                                pg[:, 64:96],
                                OP.add,
                                OP.add,
                            )
                        else:
                            nc.vector.tensor_mul(t2a[:], pg[:, 96:128], w2sb[:, 32:64])
                            nc.vector.tensor_add(t2b[:], t2a[:], pg[:, 64:96])
                    hh2 = tmp.tile([128, BL], F16, tag="hh2", name="hh2")
                    nc.scalar.activation(hh2[:], t2b[:], AF.Tanh)
                    if s == 0:
                        nc.vector.tensor_mul(h2c[:], w2sb[:, 0:32], hh2[:])
                        return
                    # Pool finishes gru2 (slack-rich, keeps DVE clear for the
                    # next step's critical ops)
                    d2 = tmp.tile([128, BL], F16, tag="d2", name="d2")
                    e2 = tmp.tile([128, BL], F16, tag="e2", name="e2")
                    nc.gpsimd.tensor_sub(d2[:], hh2[:], h2p[:])
                    nc.gpsimd.tensor_mul(e2[:], w2sb[:, 0:32], d2[:])
                    nc.gpsimd.tensor_add(h2c[:], h2p[:], e2[:])

                # schedule
                emit_xg1(0)
                for t in range(T):
                    s = t - 1
                    if t >= 1:
                        emit_rg1(t)
                    if s >= 0:
                        emit_gru2_early(s)
                    if t + 1 < T:
                        emit_xg1(t + 1)
                    if s >= 1:
                        emit_gru2_late(s)
                    emit_step(t)
                    if s >= 0:
                        emit_gru2_step(s)
                # drain gru2 for s = T-1
                emit_gru2_early(T - 1)
                emit_gru2_late(T - 1)
                emit_gru2_step(T - 1)

                # ---- dense tail ----
                pd = ps_r[T % 2]
                pd2 = ps_z[T % 2]
                h2f = sb_h2[(T - 1) % 2]
                q3 = tmp.tile([64, 32], F16, tag="q3", name="q3")
                q4 = tmp.tile([32, 32], F16, tag="q4", name="q4")
                q5 = tmp.tile([32, 32], F32, tag="q5", name="q5")
                qt = tmp.tile([32, 32], F32, tag="qt", name="qt")
                nc.vector.memset(q5[:], 0.0)
                nc.tensor.matmul(pd[0:64, 0:32], sb_wd3[:], h2f[:], start=True, stop=True)
                nc.scalar.activation(
                    q3[:], pd[0:64, 0:32], AF.Identity, bias=sb_vbd[0:64, 0:1]
                )
                nc.tensor.matmul(pd[0:32, 32:64], sb_wd4[:], q3[:], start=False, stop=True)
                nc.scalar.activation(
                    q4[:], pd[0:32, 32:64], AF.Identity, bias=sb_vbd[0:32, 1:2]
                )
                nc.tensor.matmul(pd2[0:OUT, 0:32], sb_wd5[:], q4[:], start=True, stop=True)
                nc.scalar.activation(
                    q5[0:OUT, :], pd2[0:OUT, 0:32], AF.Identity, bias=sb_vbd[0:OUT, 2:3]
                )
                nc.vector.transpose(qt[:], q5[:])
                nc.sync.dma_start(out=d_y[:], in_=qt[0:BL, 0:OUT])

    nc.compile()
    return nc


def _run(inputs, T):
    in_maps, flags = _prep(inputs, T)
    nc = _build(T, flags)
    res = run_bass_kernel_spmd(nc, in_maps, core_ids=list(range(NCORES)))
    return np.concatenate([res.results[c]["y"] for c in range(NCORES)], 0).astype(
        np.float32
    )


def kernel(**inputs):
    return _run(inputs, T_FULL)


if __name__ == "__main__":
    rng = np.random.default_rng(0)
    ins = {
        "x": rng.standard_normal((B, T_FULL, F), np.float32),
        "k1": rng.standard_normal((F, 3 * U1), np.float32) * 0.05,
        "r1": rng.standard_normal((U1, 3 * U1), np.float32) * 0.05,
        "b1": np.zeros((2, 3 * U1), np.float32),
        "k2": rng.standard_normal((U1, 3 * U2), np.float32) * 0.05,
        "r2": rng.standard_normal((U2, 3 * U2), np.float32) * 0.05,
        "b2": np.zeros((2, 3 * U2), np.float32),
        "w3": rng.standard_normal((U2, 64), np.float32) * 0.05,
        "b3": np.zeros((64,), np.float32),
        "w4": rng.standard_normal((64, 32), np.float32) * 0.05,
        "b4": np.zeros((32,), np.float32),
        "w5": rng.standard_normal((32, OUT), np.float32) * 0.05,
        "b5": np.zeros((OUT,), np.float32),
    }
    y = _run(ins, 8)
    print("ran", y.shape, y[:2, :4])


# revision 19
# speedup vs baseline: 1.1745x; 1.1745x over previous
"""Trainium2 Bass kernel for a 2-layer GRU (Keras reset_after) + 3 Dense layers.

Model (per reference):
  h1 = GRU(x; k1, r1, b1)            # [B,T,64] -> [B,T,256], full sequence
  h2 = GRU(h1; k2, r2, b2)[:, -1]    # last state, [B,128]
  y  = ((h2 @ w3 + b3) @ w4 + b4) @ w5 + b5   # [B,24]

Strategy: pure data parallel over 8 NeuronCores (batch 256 -> 32 per core).
Everything on-chip per core; transposed layout (units on partitions, batch on
the free dim) so the sequential scan needs no per-step transposes.

The scan is dependency-latency bound, so the schedule minimizes the serial
chain per step, and every PSUM accumulator gets a full 2KB bank to itself so
the framework's bank-level dependency tracking never manufactures a false
cross-step wait:
  - r gates accumulate in their own bank; a sigmoid over just the r columns
    gates the candidate path well before the z sigmoid is needed.
  - GRU2's projections that depend only on h1 (input kernel + biases) are
    emitted early; the h2-recurrent matmuls are emitted last in the PE
    stream so their wait on the previous h2 never blocks GRU1's matmuls.
  - GRU1's lerp is rewritten h' = u + w*hh with u = h - w*h computed on the
    (otherwise idle) GpSimd engine during the tanh window; only two DVE ops
    remain after each tanh.
  - h1 is written in two 32-col halves so the next step's k=0 matmuls start
    before the k=1 half lands.
  - z-gate columns of all weights are negated on the host so one sigmoid
    yields w = 1 - z directly.

All matmul weights/activations fp16 (fp32 PSUM accumulate), gate math fp32
internally on DVE/ACT with fp16 storage.
"""

import numpy as np

import concourse.bass as bass
import concourse.mybir as mybir
import concourse.tile as tile
from concourse import bacc
from concourse.bass_utils import run_bass_kernel_spmd

F16 = mybir.dt.float16
F32 = mybir.dt.float32
AF = mybir.ActivationFunctionType
OP = mybir.AluOpType

B, T_FULL, F = 256, 512, 64
U1, U2, OUT = 256, 128, 24
NCORES = 8
BL = B // NCORES  # 32 local batch


def _prep(inputs, T):
    """Host-side preprocessing -> (list of per-core input dicts, flags)."""
    x = np.asarray(inputs["x"], np.float32)[:, :T, :]
    k1 = np.asarray(inputs["k1"], np.float32)
    r1 = np.asarray(inputs["r1"], np.float32)
    b1 = np.asarray(inputs["b1"], np.float32)
    k2 = np.asarray(inputs["k2"], np.float32)
    r2 = np.asarray(inputs["r2"], np.float32)
    b2 = np.asarray(inputs["b2"], np.float32)
    w3 = np.asarray(inputs["w3"], np.float32)
    b3 = np.asarray(inputs["b3"], np.float32)
    w4 = np.asarray(inputs["w4"], np.float32)
    b4 = np.asarray(inputs["b4"], np.float32)
    w5 = np.asarray(inputs["w5"], np.float32)
    b5 = np.asarray(inputs["b5"], np.float32)

    s1 = np.ones(3 * U1, np.float32)
    s1[:U1] = -1.0  # z-gate negation
    s2 = np.ones(3 * U2, np.float32)
    s2[:U2] = -1.0

    k1e = k1 * s1
    r1e = r1 * s1
    b1e = (b1[0] + b1[1]) * s1  # only used for z/r columns
    k2e = k2 * s2
    r2e = r2 * s2
    b2zr = ((b2[0] + b2[1]) * s2)[: 2 * U2]

    # k1 z/r part augmented with bias row: [65, 512]
    wk1zr = np.concatenate([k1e[:, : 2 * U1], b1e[None, : 2 * U1]], 0)
    # k1 h part augmented with input-bias row: [65, 256]
    wk1h = np.concatenate([k1[:, 2 * U1 :], b1[0][None, 2 * U1 :]], 0)
    # r1 tiles: tile (m,k) at cols (m*2+k)*128
    wr1 = r1e.reshape(2, 128, 6, 128).transpose(1, 2, 0, 3).reshape(128, 12 * 128)
    # k2 tiles: tile (m,k) at cols (m*2+k)*128
    wk2 = k2e.reshape(2, 128, 3, 128).transpose(1, 2, 0, 3).reshape(128, 6 * 128)
    wr2 = r2e  # [128, 384], tile m at m*128

    vb1h = np.stack([b1[1, 2 * U1 : 2 * U1 + 128], b1[1, 2 * U1 + 128 :]], 1)  # [128,2]
    vb2h = np.stack([b2[0, 2 * U2 :], b2[1, 2 * U2 :]], 1)  # [128,2]
    vbd = np.zeros((128, 3), np.float32)
    vbd[:64, 0] = b3
    vbd[:32, 1] = b4
    vbd[:OUT, 2] = b5

    flags = {
        "HAS_B1H": bool(np.any(b1[1, 2 * U1 :] != 0)),
        "HAS_B2H": bool(np.any(b2[:, 2 * U2 :] != 0)),
    }

    shared = {
        "wk1zr": wk1zr.astype(np.float16),
        "wk1h": wk1h.astype(np.float16),
        "wr1": wr1.astype(np.float16),
        "wk2": wk2.astype(np.float16),
        "wr2": wr2.astype(np.float16),
        "wb2zr": b2zr[None, :].astype(np.float16),
        "vb1h": vb1h.astype(np.float32),
        "vb2h": vb2h.astype(np.float32),
        "vbd": vbd.astype(np.float32),
        "wd3": w3.astype(np.float16),
        "wd4": w4.astype(np.float16),
        "wd5": w5.astype(np.float16),
    }

    in_maps = []
    for c in range(NCORES):
        xs = x[c * BL : (c + 1) * BL]  # [BL, T, F]
        xt = np.ascontiguousarray(xs.transpose(2, 1, 0)).reshape(F, T * BL)
        xin = np.concatenate([xt, np.ones((1, T * BL), np.float32)], 0)
        m = dict(shared)
        m["xin"] = xin.astype(np.float16)
        in_maps.append(m)
    return in_maps, flags


def _build(T, flags):
    """Emit the Bass program for T timesteps. Returns compiled nc."""
    HAS_B1H = flags["HAS_B1H"]
    HAS_B2H = flags["HAS_B2H"]
    nc = bacc.Bacc("TRN2", target_bir_lowering=False, debug=False, num_devices=NCORES)

    d_xin = nc.dram_tensor("xin", [F + 1, T * BL], F16, kind="ExternalInput").ap()
    d_wk1zr = nc.dram_tensor("wk1zr", [F + 1, 512], F16, kind="ExternalInput").ap()
    d_wk1h = nc.dram_tensor("wk1h", [F + 1, 256], F16, kind="ExternalInput").ap()
    d_wr1 = nc.dram_tensor("wr1", [128, 1536], F16, kind="ExternalInput").ap()
    d_wk2 = nc.dram_tensor("wk2", [128, 768], F16, kind="ExternalInput").ap()
    d_wr2 = nc.dram_tensor("wr2", [128, 384], F16, kind="ExternalInput").ap()
    d_wb2zr = nc.dram_tensor("wb2zr", [1, 256], F16, kind="ExternalInput").ap()
    d_vb1h = nc.dram_tensor("vb1h", [128, 2], F32, kind="ExternalInput").ap()
    d_vb2h = nc.dram_tensor("vb2h", [128, 2], F32, kind="ExternalInput").ap()
    d_vbd = nc.dram_tensor("vbd", [128, 3], F32, kind="ExternalInput").ap()
    d_wd3 = nc.dram_tensor("wd3", [128, 64], F16, kind="ExternalInput").ap()
    d_wd4 = nc.dram_tensor("wd4", [64, 32], F16, kind="ExternalInput").ap()
    d_wd5 = nc.dram_tensor("wd5", [32, OUT], F16, kind="ExternalInput").ap()
    d_y = nc.dram_tensor("y", [BL, OUT], F32, kind="ExternalOutput").ap()

    with tile.TileContext(nc) as tc:
        with (
            tc.tile_pool(name="big", bufs=1) as big,
            tc.tile_pool(name="wts", bufs=1) as wts,
            tc.tile_pool(name="state", bufs=1) as state,
            tc.tile_pool(name="tmp", bufs=3) as tmp,
        ):
            sb_x = big.tile([F + 1, T * BL], F16, tag="sb_x", name="sb_x")
            # flat [128, T*64] so the per-step slice is a clean 2-D AP (a 3-D
            # AP on the DVE costs ~2x on the critical-path add)
            sb_xg1h = big.tile([128, T * 64], F16, tag="sb_xg1h", name="sb_xg1h")
            xg1h_3d = sb_xg1h.rearrange("p (t c) -> p t c", c=64)

            def wtile(name, shape, dt, src):
                t_ = wts.tile(shape, dt, tag=name, name=name)
                nc.sync.dma_start(out=t_[:], in_=src[:])
                return t_

            sb_wk1zr = wtile("sb_wk1zr", [F + 1, 512], F16, d_wk1zr)
            sb_wk1h = wtile("sb_wk1h", [F + 1, 256], F16, d_wk1h)
            sb_wr1 = wtile("sb_wr1", [128, 1536], F16, d_wr1)
            sb_wk2 = wtile("sb_wk2", [128, 768], F16, d_wk2)
            sb_wr2 = wtile("sb_wr2", [128, 384], F16, d_wr2)
            sb_wb2zr = wtile("sb_wb2zr", [1, 256], F16, d_wb2zr)
            sb_vb1h = wtile("sb_vb1h", [128, 2], F32, d_vb1h)
            sb_vb2h = wtile("sb_vb2h", [128, 2], F32, d_vb2h)
            sb_vbd = wtile("sb_vbd", [128, 3], F32, d_vbd)
            sb_wd3 = wtile("sb_wd3", [128, 64], F16, d_wd3)
            sb_wd4 = wtile("sb_wd4", [64, 32], F16, d_wd4)
            sb_wd5 = wtile("sb_wd5", [32, OUT], F16, d_wd5)

            sb_ones = wts.tile([1, BL], F16, tag="sb_ones", name="sb_ones")
            nc.vector.memset(sb_ones[:], 1.0)

            # x load, split across a few DMAs
            nchunk = 4
            cw = (T * BL) // nchunk
            for i in range(nchunk):
                nc.sync.dma_start(
                    out=sb_x[:, i * cw : (i + 1) * cw],
                    in_=d_xin[:, i * cw : (i + 1) * cw],
                )

            # ---- bulk precompute xg1h = [x;1] @ [k1_h; b1_0h]  -> sb_xg1h ----
            with tc.tile_pool(name="bulkps", bufs=4, space="PSUM") as bulkps:
                CH = 16  # timesteps per matmul (N = CH*BL = 512)
                for ci in range((T + CH - 1) // CH):
                    t0 = ci * CH
                    ts_ = min(CH, T - t0)
                    n = ts_ * BL
                    for m in range(2):
                        pb = bulkps.tile([128, 512], F32, tag="pb", name="pb")
                        nc.tensor.matmul(
                            pb[:, :n],
                            sb_wk1h[:, m * 128 : (m + 1) * 128],
                            sb_x[:, t0 * BL : t0 * BL + n],
                            start=True,
                            stop=True,
                        )
                        dst = xg1h_3d[:, t0 : t0 + ts_, m * 32 : (m + 1) * 32]
                        src = pb.rearrange("p (t b) -> p t b", b=BL)[:, :ts_, :]
                        if m == 0:
                            nc.vector.tensor_copy(dst, src)
                        else:
                            nc.scalar.copy(dst, src)

            # ---- the scan ----
            # Eight PSUM accumulators, each padded to a full 2KB bank so the
            # framework's dependency tracking (bank-granular) never couples
            # them:
            #   ps_r[i]  cols 0:64   gru1 r gates (m=2 -> 0:32, m=3 -> 32:64)
            #   ps_z[i]  cols 0:64   gru1 z gates (m=0 -> 0:32, m=1 -> 32:64)
            #   ps_h[i]  cols 0:64   gru1 candidate (m=4 -> 0:32, m=5 -> 32:64)
            #   ps_g[i]  cols 0:64 gru2 z/r gates; 64:96 xh2; 96:128 rh2
            with tc.tile_pool(name="ps", bufs=1, space="PSUM") as psp:
                def pbank(nm):
                    return [
                        psp.tile([128, 512], F32, tag=f"{nm}_{i}", name=f"{nm}_{i}")
                        for i in range(2)
                    ]

                ps_r = pbank("ps_r")
                ps_z = pbank("ps_z")
                ps_h = pbank("ps_h")
                ps_g = pbank("ps_g")
                sb_h1 = [
                    state.tile([128, 64], F16, tag=f"sb_h1_{i}", name=f"sb_h1_{i}")
                    for i in range(2)
                ]
                sb_h2 = [
                    state.tile([128, BL], F16, tag=f"sb_h2_{i}", name=f"sb_h2_{i}")
                    for i in range(2)
                ]

                def emit_xg1(s):
                    """x-side z/r projections (+biases) for step s; starts the
                    accumulation groups for the r and z banks of step s."""
                    rhs = sb_x[:, s * BL : (s + 1) * BL]
                    for m in (2, 3):  # r tiles
                        nc.tensor.matmul(
                            ps_r[s % 2][:, (m - 2) * 32 : (m - 1) * 32],
                            sb_wk1zr[:, m * 128 : (m + 1) * 128],
                            rhs,
                            start=(m == 2),
                            stop=(s == 0),
                        )
                    for m in (0, 1):  # z tiles
                        nc.tensor.matmul(
                            ps_z[s % 2][:, m * 32 : (m + 1) * 32],
                            sb_wk1zr[:, m * 128 : (m + 1) * 128],
                            rhs,
                            start=(m == 0),
                            stop=(s == 0),
                        )

                def emit_rg1(t):
                    """Recurrent projections for gru1 step t: r gates first
                    (k-major so the k=0 half of h1 unblocks the first pairs),
                    then z gates, then the candidate (h) tiles."""
                    h1p = sb_h1[(t - 1) % 2]
                    for k in range(2):
                        for m in (2, 3):
                            nc.tensor.matmul(
                                ps_r[t % 2][:, (m - 2) * 32 : (m - 1) * 32],
                                sb_wr1[:, (m * 2 + k) * 128 : (m * 2 + k + 1) * 128],
                                h1p[:, k * 32 : (k + 1) * 32],
                                start=False,
                                stop=(k == 1),
                            )
                    for k in range(2):
                        for m in (0, 1):
                            nc.tensor.matmul(
                                ps_z[t % 2][:, m * 32 : (m + 1) * 32],
                                sb_wr1[:, (m * 2 + k) * 128 : (m * 2 + k + 1) * 128],
                                h1p[:, k * 32 : (k + 1) * 32],
                                start=False,
                                stop=(k == 1),
                            )
                    for k in range(2):
                        for i, m in enumerate((4, 5)):
                            nc.tensor.matmul(
                                ps_h[t % 2][:, i * 32 : (i + 1) * 32],
                                sb_wr1[:, (m * 2 + k) * 128 : (m * 2 + k + 1) * 128],
                                h1p[:, k * 32 : (k + 1) * 32],
                                start=(k == 0 and i == 0),
                                stop=(k == 1),
                            )

                def emit_gru2_early(s):
                    """gru2 projections for step s that depend only on h1[s]:
                    input-kernel z/r, biases, and xh2.  Starts the ps_g group."""
                    pg = ps_g[s % 2]
                    h1s = sb_h1[s % 2]
                    for m in range(2):  # z, r gates
                        reg = pg[:, m * 32 : (m + 1) * 32]
                        for k in range(2):
                            nc.tensor.matmul(
                                reg,
                                sb_wk2[:, (m * 2 + k) * 128 : (m * 2 + k + 1) * 128],
                                h1s[:, k * 32 : (k + 1) * 32],
                                start=(m == 0 and k == 0),
                                stop=False,
                            )
                        nc.tensor.matmul(
                            reg,
                            sb_wb2zr[:, m * 128 : (m + 1) * 128],
                            sb_ones[:],
                            start=False,
                            stop=(s == 0),
                        )
                    for k in range(2):  # xh2
                        nc.tensor.matmul(
                            pg[:, 64:96],
                            sb_wk2[:, (4 + k) * 128 : (5 + k) * 128],
                            h1s[:, k * 32 : (k + 1) * 32],
                            start=False,
                            stop=(k == 1),
                        )

                def emit_gru2_late(s):
                    """gru2 recurrent projections for step s (need h2[s-1]);
                    emitted last in the PE stream so their wait can't block
                    the next step's gru1 matmuls."""
                    pg = ps_g[s % 2]
                    h2p = sb_h2[(s - 1) % 2]
                    for m in range(2):
                        nc.tensor.matmul(
                            pg[:, m * 32 : (m + 1) * 32],
                            sb_wr2[:, m * 128 : (m + 1) * 128],
                            h2p[:],
                            start=False,
                            stop=True,
                        )
                    nc.tensor.matmul(
                        pg[:, 96:128],
                        sb_wr2[:, 256:384],
                        h2p[:],
                        start=False,
                        stop=True,
                    )

                zb_holder = [None]

                # Per step t (s = t-1 is the gru2 step) the engines run:
                #   ACT:  sig_r(t), sig_z(t), tanh1(t), sig_g2(s), tanh2(s)
                #   DVE:  t1, pre1, t2a, t2b, h1c, v2, h2c   (PSUM-capable)
                #   Pool: wh, u, v, wh2, u2                  (SBUF-only prep)
                def emit_step(t):
                    s = t - 1
                    h1p = sb_h1[(t - 1) % 2]
                    h1c = sb_h1[t % 2]

                    # -- ACT: gru1 sigmoids --
                    r1sb = tmp.tile([128, 64], F16, tag="r1sb", name="r1sb")
                    w1sb = tmp.tile([128, 64], F16, tag="w1sb", name="w1sb")
                    if t >= 1:
                        nc.scalar.activation(r1sb[:], ps_r[t % 2][:, 0:64], AF.Sigmoid)
                    nc.scalar.activation(w1sb[:], ps_z[t % 2][:, 0:64], AF.Sigmoid)

                    # -- gru1 candidate + combine --
                    hh1 = tmp.tile([128, 64], F16, tag="hh1", name="hh1")
                    if t == 0:
                        nc.scalar.activation(hh1[:], sb_xg1h[:, 0:64], AF.Tanh)
                        nc.vector.tensor_mul(h1c[:], w1sb[:], hh1[:])
                        return
                    t1b = tmp.tile([128, 64], F16, tag="t1b", name="t1b")
                    ph = ps_h[t % 2]
                    if HAS_B1H:
                        for i in range(2):
                            nc.vector.scalar_tensor_tensor(
                                t1b[:, i * 32 : (i + 1) * 32],
                                ph[:, i * 32 : (i + 1) * 32],
                                sb_vb1h[:, i : i + 1],
                                r1sb[:, i * 32 : (i + 1) * 32],
                                OP.add,
                                OP.mult,
                            )
                    else:
                        nc.vector.tensor_mul(t1b[:], ph[:, 0:64], r1sb[:])
                    pre1 = tmp.tile([128, 64], F16, tag="pre1", name="pre1")
                    nc.vector.tensor_add(
                        pre1[:], t1b[:], sb_xg1h[:, t * 64 : (t + 1) * 64]
                    )
                    # zero [128,1] written after pre1; sig_g2 takes it as its
                    # bias operand purely to pin its schedule slot after tanh1
                    # (the scheduler's cost model underestimates gru2's path
                    # and would otherwise run sig_g2 first on ACT, delaying
                    # tanh1 by ~400ns).
                    zb = tmp.tile([128, 1], F32, tag="zb", name="zb")
                    nc.vector.tensor_scalar_mul(zb[:], pre1[:, 0:1], 0.0)
                    zb_holder[0] = zb
                    nc.scalar.activation(hh1[:], pre1[:], AF.Tanh)

                    # u = (1-w)*h1p on the DVE itself, filling the tanh window
                    # (Pool is ~2x slower and contends for the SBUF port), then
                    # only two ops remain after the tanh: h1c = u + w*hh1.
                    wh = tmp.tile([128, 64], F16, tag="wh", name="wh")
                    u = tmp.tile([128, 64], F16, tag="u", name="u")
                    nc.vector.tensor_mul(wh[:], w1sb[:], h1p[:])
                    nc.vector.tensor_sub(u[:], h1p[:], wh[:])
                    v = tmp.tile([128, 64], F16, tag="v", name="v")
                    nc.vector.tensor_mul(v[:], w1sb[:], hh1[:])
                    nc.vector.tensor_add(h1c[:], u[:], v[:])

                def emit_gru2_step(s):
                    """gru2 elementwise for step s (ACT sig/tanh, DVE psum-side
                    ops + combine, Pool u2 prep)."""
                    pg = ps_g[s % 2]
                    h2p = sb_h2[(s - 1) % 2] if s > 0 else None
                    h2c = sb_h2[s % 2]
                    w2sb = tmp.tile([128, 64], F16, tag="w2sb", name="w2sb")
                    if zb_holder[0] is not None:
                        nc.scalar.activation(
                            w2sb[:], pg[:, 0:64], AF.Sigmoid, bias=zb_holder[0][:, 0:1]
                        )
                    else:
                        nc.scalar.activation(w2sb[:], pg[:, 0:64], AF.Sigmoid)

                    t2b = tmp.tile([128, BL], F16, tag="t2b", name="t2b")
                    if s == 0:
                        nc.vector.tensor_scalar_add(
                            t2b[:], pg[:, 64:96], sb_vb2h[:, 0:1]
                        )
                    else:
                        t2a = tmp.tile([128, BL], F16, tag="t2a", name="t2a")
                        if HAS_B2H:
                            nc.vector.scalar_tensor_tensor(
                                t2a[:],
                                pg[:, 96:128],
                                sb_vb2h[:, 1:2],
                                w2sb[:, 32:64],
                                OP.add,
                                OP.mult,
                            )
                            nc.vector.scalar_tensor_tensor(
                                t2b[:],
                                t2a[:],
                                sb_vb2h[:, 0:1],
                                pg[:, 64:96],
                                OP.add,
                                OP.add,
                            )
                        else:
                            nc.vector.tensor_mul(t2a[:], pg[:, 96:128], w2sb[:, 32:64])
                            nc.vector.tensor_add(t2b[:], t2a[:], pg[:, 64:96])
                    if s >= 1:
                        # Pool preps u2 = (1-w2)*h2p during the t2a/t2b window
                        wh2 = tmp.tile([128, BL], F16, tag="wh2", name="wh2")
                        u2 = tmp.tile([128, BL], F16, tag="u2", name="u2")
                        nc.gpsimd.tensor_mul(wh2[:], w2sb[:, 0:32], h2p[:])
                        nc.gpsimd.tensor_sub(u2[:], h2p[:], wh2[:])
                    hh2 = tmp.tile([128, BL], F16, tag="hh2", name="hh2")
                    nc.scalar.activation(hh2[:], t2b[:], AF.Tanh)
                    if s == 0:
                        nc.vector.tensor_mul(h2c[:], w2sb[:, 0:32], hh2[:])
                        return
                    # only two Pool ops after the tanh: h2c = u2 + w2*hh2
                    v2 = tmp.tile([128, BL], F16, tag="v2", name="v2")
                    nc.gpsimd.tensor_mul(v2[:], w2sb[:, 0:32], hh2[:])
                    nc.gpsimd.tensor_add(h2c[:], u2[:], v2[:])

                # schedule
                emit_xg1(0)
                for t in range(T):
                    s = t - 1
                    if t >= 1:
                        emit_rg1(t)
                    if s >= 0:
                        emit_gru2_early(s)
                    if t + 1 < T:
                        emit_xg1(t + 1)
                    if s >= 1:
                        emit_gru2_late(s)
                    emit_step(t)
                    if s >= 0:
                        emit_gru2_step(s)
                # drain gru2 for s = T-1
                emit_gru2_early(T - 1)
                emit_gru2_late(T - 1)
                emit_gru2_step(T - 1)

                # ---- dense tail ----
                pd = ps_r[T % 2]
                pd2 = ps_z[T % 2]
                h2f = sb_h2[(T - 1) % 2]
                q3 = tmp.tile([64, 32], F16, tag="q3", name="q3")
                q4 = tmp.tile([32, 32], F16, tag="q4", name="q4")
                q5 = tmp.tile([32, 32], F32, tag="q5", name="q5")
                qt = tmp.tile([32, 32], F32, tag="qt", name="qt")
                nc.vector.memset(q5[:], 0.0)
                nc.tensor.matmul(pd[0:64, 0:32], sb_wd3[:], h2f[:], start=True, stop=True)
                nc.scalar.activation(
                    q3[:], pd[0:64, 0:32], AF.Identity, bias=sb_vbd[0:64, 0:1]
                )
                nc.tensor.matmul(pd[0:32, 32:64], sb_wd4[:], q3[:], start=False, stop=True)
                nc.scalar.activation(
                    q4[:], pd[0:32, 32:64], AF.Identity, bias=sb_vbd[0:32, 1:2]
                )
                nc.tensor.matmul(pd2[0:OUT, 0:32], sb_wd5[:], q4[:], start=True, stop=True)
                nc.scalar.activation(
                    q5[0:OUT, :], pd2[0:OUT, 0:32], AF.Identity, bias=sb_vbd[0:OUT, 2:3]
                )
                nc.vector.transpose(qt[:], q5[:])
                nc.sync.dma_start(out=d_y[:], in_=qt[0:BL, 0:OUT])

    nc.compile()
    return nc


def _run(inputs, T):
    in_maps, flags = _prep(inputs, T)
    nc = _build(T, flags)
    res = run_bass_kernel_spmd(nc, in_maps, core_ids=list(range(NCORES)))
    return np.concatenate([res.results[c]["y"] for c in range(NCORES)], 0).astype(
        np.float32
    )


def kernel(**inputs):
    return _run(inputs, T_FULL)


if __name__ == "__main__":
    rng = np.random.default_rng(0)
    ins = {
        "x": rng.standard_normal((B, T_FULL, F), np.float32),
        "k1": rng.standard_normal((F, 3 * U1), np.float32) * 0.05,
        "r1": rng.standard_normal((U1, 3 * U1), np.float32) * 0.05,
        "b1": np.zeros((2, 3 * U1), np.float32),
        "k2": rng.standard_normal((U1, 3 * U2), np.float32) * 0.05,
        "r2": rng.standard_normal((U2, 3 * U2), np.float32) * 0.05,
        "b2": np.zeros((2, 3 * U2), np.float32),
        "w3": rng.standard_normal((U2, 64), np.float32) * 0.05,
        "b3": np.zeros((64,), np.float32),
        "w4": rng.standard_normal((64, 32), np.float32) * 0.05,
        "b4": np.zeros((32,), np.float32),
        "w5": rng.standard_normal((32, OUT), np.float32) * 0.05,
        "b5": np.zeros((OUT,), np.float32),
    }
    y = _run(ins, 8)
    print("ran", y.shape, y[:2, :4])
